# revision 29
# baseline (speedup 1.0000x reference)
"""MHA kernel for Trainium2, 8-core tensor-parallel (2 heads per core).

Problem (hardcoded): x [2, 2048, 1024] fp32, Wq/Wk/Wv/Wo [1024, 1024],
bq/bk/bv/bo [1024], H=16 heads, DH=64.  out = MHA(x).

Sharding: heads are split 8 ways (2 heads = 128 proj columns per core).
Each core computes its heads' attention output and a partial output
projection (row-parallel Wo); the host sums the 8 partials and adds the
closed-form bias terms (bv @ Wo + bo).

Structure: the kernel is ScalarE-bound (softmax exp = 16.8M elems/core
at 1 elem/cycle ~= 147us).  Everything else is emitted as a software
pipeline around the exp stream so the PE never idles in lumps (HAM
throttle) and the QKV/V projections ride in the PE's spare cycles
instead of a serial prologue:

  per global k-tile step s (combo c = s//16, k = s%16):
    - background proj work (kt/qt chunks, V token-tiles) per a static
      schedule with just-in-time deadlines
    - scores pair for (c, k): two K=64 matmuls on PE row-groups 0-1 /
      2-3 (run concurrently), exp on ScalarE -> pt (bf16)
    - AV accumulation steps lagged LAG behind scores (h1 one more);
      ones-column in V gives the softmax denominator in PSUM row 64
    - at combo boundaries: oraw copy, one [1,1024] Ln + Exp for both
      heads' reciprocals, gpsimd broadcast, normalize, then the
      output projection spread 2 MMs/step with bf16 staging + DMA out.
"""

import numpy as np
import ml_dtypes

D = 1024
T = 4096          # B*S tokens
S = 2048
B = 2
NH = 2            # heads per core
DH = 64
NCORES = 8
SCALE = 0.125     # 1/sqrt(DH)
NKT = S // 128    # 16 key tiles per batch
NQC = S // 512    # 4 query chunks per batch
NCK = T // 512    # 8 x^T column chunks
VSLOT = DH + 1    # 65: [V columns | ones column]
NCOMBO = B * NQC  # 8
LAG = 5           # AV trails scores by LAG k-tile steps

_CACHE = {}


def _build_nc(reps=1):
    import concourse.bacc as bacc
    import concourse.mybir as mybir
    import concourse.tile as tile
    from concourse.hw_specs import get_activation_tables as _gat

    # Pin Exp and Ln to the one table set that holds both, so the
    # table-load placement pass emits a single ACT_TABLE_LOAD instead of
    # thrashing between exp_and_others and natural_log every combo.
    def _pinned_tables(arch):
        out = {}
        for k, fns in _gat(arch).items():
            if k != "natural_log_exp_and_others":
                fns = {f for f in fns if f.name not in ("Exp", "Ln")}
            out[k] = fns
        return out
    bacc.get_activation_tables = _pinned_tables

    dt = mybir.dt
    f32, bf16 = dt.float32, dt.bfloat16

    nc = bacc.Bacc("TRN2", target_bir_lowering=False, debug=False,
                   num_devices=NCORES)

    xT = nc.dram_tensor("xT", [D, T], bf16, kind="ExternalInput")
    wq_d = nc.dram_tensor("wq", [D, 128], bf16, kind="ExternalInput")
    wk_d = nc.dram_tensor("wk", [D, 128], bf16, kind="ExternalInput")
    wv_d = nc.dram_tensor("wv", [D, 128], bf16, kind="ExternalInput")
    wo_d = nc.dram_tensor("wo", [128, D], bf16, kind="ExternalInput")
    bq_d = nc.dram_tensor("bq", [128, 1], f32, kind="ExternalInput")
    bk_d = nc.dram_tensor("bk", [128, 1], f32, kind="ExternalInput")
    outp = nc.dram_tensor("outp", [T, D], bf16, kind="ExternalOutput")

    with tile.TileContext(nc) as tc:
      for _rep in range(reps):
        with (
            tc.tile_pool(name="persist", bufs=1) as pp,
            tc.tile_pool(name="pt", bufs=2) as ptp,
            tc.tile_pool(name="onorm", bufs=2) as onp,
            tc.tile_pool(name="oraw", bufs=2) as orp,
            tc.tile_pool(name="recip", bufs=2) as rcp,
            tc.tile_pool(name="outsb", bufs=3) as osp,
            tc.tile_pool(name="st_ps", bufs=2, space="PSUM") as stp,
            tc.tile_pool(name="av_ps", bufs=2, space="PSUM") as avp,
            tc.tile_pool(name="op_ps", bufs=2, space="PSUM") as opp,
        ):
            # ---- weights / biases / x^T d-tiles ----
            # DMA issue on the sync engine is ~650ns per dma_start, so
            # order and granularity matter: x batch-0 chunk 0 first (it
            # gates the whole pipeline), then the weights the prologue
            # needs, then coarse transfers for the rest.
            wq = pp.tile([128, D], bf16, tag="wq")
            wk = pp.tile([128, D], bf16, tag="wk")
            wv = pp.tile([128, D], bf16, tag="wv")
            wo = pp.tile([128, D], bf16, tag="wo")
            bq = pp.tile([128, 1], f32, tag="bq")
            bk = pp.tile([128, 1], f32, tag="bk")
            xt_all = pp.tile([128, 8 * T], bf16, tag="xt")
            xt3 = xt_all.rearrange("p (d t) -> p d t", d=8)
            xt = [xt3[:, d, :] for d in range(8)]
            # Per-d-tile DMAs parallelize across HW queues (a single
            # big DMA runs on one queue at ~150GB/s).  The scalar
            # queue carries NO DMA issues: each dma_start costs ~650ns
            # of queue time and would delay the first exp — the exp
            # stream IS the kernel bottleneck.  Weights ride the
            # (otherwise idle) gpsimd software-DGE; all x loads on sync.
            for w_sb, w_dr in ((wk, wk_d), (wq, wq_d), (wv, wv_d)):
                nc.gpsimd.dma_start(
                    out=w_sb.rearrange("p (t c) -> p t c", c=128),
                    in_=w_dr.ap().rearrange("(t p) c -> p t c", p=128),
                )
            nc.gpsimd.dma_start(out=bk[:, :], in_=bk_d.ap()[:, :])
            nc.gpsimd.dma_start(out=bq[:, :], in_=bq_d.ap()[:, :])
            nc.gpsimd.dma_start(out=wo[:, :], in_=wo_d.ap()[:, :])
            for d in range(8):          # b0 chunk 0 (gates prologue)
                nc.sync.dma_start(out=xt3[:, d, 0:512],
                                  in_=xT.ap()[d * 128:(d + 1) * 128, 0:512])
            for d in range(8):          # rest of b0, 3KB lines
                nc.sync.dma_start(
                    out=xt3[:, d, 512:2048],
                    in_=xT.ap()[d * 128:(d + 1) * 128, 512:2048])
            for d in range(8):          # b1, 4KB lines
                nc.sync.dma_start(out=xt3[:, d, 2048:4096],
                                  in_=xT.ap()[d * 128:(d + 1) * 128, 2048:4096])

            qt = pp.tile([128, T], bf16, tag="qt")
            kt = pp.tile([128, T], bf16, tag="kt")
            wq3 = wq.rearrange("p (t c) -> p t c", c=128)
            wk3 = wk.rearrange("p (t c) -> p t c", c=128)
            wv3 = wv.rearrange("p (t c) -> p t c", c=128)

            vtm = []
            for b in range(B):
                v_sb = pp.tile([128, NH * NKT * VSLOT], bf16, tag=f"v{b}")
                v4 = v_sb.rearrange("p (h k c) -> p h k c", h=NH, k=NKT)
                nc.vector.memset(v4[:, :, :, DH:DH + 1], 1.0)
                vtm.append(v_sb)

            # ---- background-work emitters (projections) ----
            _chunk_ps = {}
            _chunk_seq = [0]

            def emit_proj_half(proj_sb, w3, b_sb, nck, half):
                # half a 512-col chunk of Q^T/K^T (4 of 8 d-steps) —
                # finer background quanta so a chunk never overflows
                # one k-tile step's PE slack and stalls the exp stream
                cs = slice(nck * 512, (nck + 1) * 512)
                key = (id(proj_sb), nck)
                if half == 0:
                    _chunk_seq[0] += 1
                    _chunk_ps[key] = opp.tile([128, 512], f32, tag="op",
                                              name=f"pch{_chunk_seq[0]}")
                ps = _chunk_ps[key]
                for d in range(4 * half, 4 * half + 4):
                    nc.tensor.matmul(
                        ps[:, :], w3[:, d, :], xt[d][:, cs],
                        start=(d == 0), stop=(d == 7),
                    )
                if half == 1:
                    nc.vector.tensor_scalar_add(
                        proj_sb[:, cs], ps[:, :], b_sb[:, :])
                    del _chunk_ps[key]

            def emit_proj_chunk(proj_sb, w3, b_sb, nck):
                emit_proj_half(proj_sb, w3, b_sb, nck, 0)
                emit_proj_half(proj_sb, w3, b_sb, nck, 1)

            def emit_v_tile(b, kti):
                # one token-major V tile [128 tok, 2x64] for batch b
                v4 = vtm[b].rearrange("p (h k c) -> p h k c", h=NH, k=NKT)
                tok0 = b * S + kti * 128
                ps = opp.tile([128, 128], f32, tag="op")
                for d in range(8):
                    nc.tensor.matmul(
                        ps[:, :], xt[d][:, tok0:tok0 + 128], wv3[:, d, :],
                        start=(d == 0), stop=(d == 7),
                    )
                nc.vector.tensor_copy(
                    v4[:, :, kti, 0:DH],
                    ps.rearrange("p (h c) -> p h c", h=NH)[:, :, :],
                )

            def bg_item(kind, a, bb):
                if kind == "kt":
                    emit_proj_chunk(kt, wk3, bk, a)
                elif kind == "qt":
                    emit_proj_chunk(qt, wq3, bq, a)
                elif kind == "kth":
                    emit_proj_half(kt, wk3, bk, a, bb)
                elif kind == "qth":
                    emit_proj_half(qt, wq3, bq, a, bb)
                else:
                    emit_v_tile(bb, a)

            # static schedule: step -> background item.  Deadlines:
            # scores(c,k) needs kt chunk (4b + k//4) and qt chunk
            # (4b + qc) by step 16c+k; av(c,k) at step 16c+k+LAG needs
            # v(b,k).  At most ONE item per step and none at k<3
            # (except combo 0) so the in-order PE never delays a
            # scores pair by more than the pipeline slack -> the exp
            # stream on ScalarE (the bottleneck) never starves.
            bg = {}
            def at(step, kind, a, bb=0):
                assert step not in bg, step
                bg[step] = (kind, a, bb)
            # Slots avoid outproj steps (k 10-13) and combo boundaries
            # (k 0-2) so a background chunk never delays a scores pair
            # past the exp stream's ~0.8us of ring slack.  Exceptions
            # are small v-tiles whose AV deadlines force early steps.
            at(1, "kt", 1); at(4, "kt", 2); at(7, "kt", 3)
            at(14, "qt", 1)                      # b0 qc1, due 16
            vi = 0
            for s in list(range(16)) + [16, 17, 18, 19]:
                if s not in bg and vi < NKT:
                    at(s, "v", vi, 0); vi += 1    # v(b0,k) due k+LAG
            at(20, "qth", 2, 0); at(21, "qth", 2, 1)   # b0 qc2, due 32
            at(35, "kth", 4, 0); at(36, "kth", 4, 1)   # kt b1 j0, due 64
            at(37, "kth", 5, 0); at(38, "kth", 5, 1)   # j1, due 68
            at(39, "qth", 3, 0); at(40, "qth", 3, 1)   # b0 qc3, due 48
            at(41, "v", 0, 1); at(46, "v", 1, 1); at(47, "v", 2, 1)
            at(51, "kth", 6, 0); at(52, "kth", 6, 1)   # j2, due 72
            at(53, "kth", 7, 0); at(54, "kth", 7, 1)   # j3, due 76
            at(55, "qth", 4, 0); at(56, "qth", 4, 1)   # b1 qc0, due 64
            at(57, "v", 3, 1); at(62, "v", 4, 1); at(63, "v", 5, 1)
            at(67, "qth", 5, 0); at(68, "qth", 5, 1)   # b1 qc1, due 80
            for i, s in enumerate((69, 70, 71, 72, 73, 78, 79)):
                at(s, "v", 6 + i, 1)             # v b1 6-12, due 75+
            at(81, "v", 13, 1); at(82, "v", 14, 1)     # due 82/83
            at(83, "v", 15, 1)                   # due 84
            at(84, "qth", 6, 0); at(85, "qth", 6, 1)   # due 96
            at(99, "qth", 7, 0); at(100, "qth", 7, 1)  # due 112
            # prologue: minimum to start combo 0
            emit_proj_chunk(kt, wk3, bk, 0)
            emit_proj_chunk(qt, wq3, bq, 0)

            # ---- attention pipeline state ----
            combos = [(b, qc) for b in range(B) for qc in range(NQC)]
            pt_tiles = [None] * NCOMBO          # [128, NH*NKT*512] bf16
            av_tiles = [[None, None] for _ in range(NCOMBO)]
            oraw_tiles = [None] * NCOMBO        # [VSLOT, 1024] f32 SBUF
            onorm_tiles = [None] * NCOMBO

            def emit_scores_exp(c, k):
                b, qc = combos[c]
                q0 = b * S + qc * 512
                k0 = b * S + k * 128
                if k == 0:
                    pt_tiles[c] = ptp.tile([128, NH * NKT * 512], bf16,
                                           tag="pt", name=f"pt{c}")
                pt3 = pt_tiles[c].rearrange("p (h k q) -> p h k q",
                                            h=NH, k=NKT)
                st = stp.tile([128, 1024], f32, tag="st")
                for h in range(NH):
                    hp = h * DH
                    nc.tensor.matmul(
                        st[:, h * 512:(h + 1) * 512],
                        kt[hp:hp + DH, k0:k0 + 128],
                        qt[hp:hp + DH, q0:q0 + 512],
                        start=True, stop=True,
                    )
                nc.scalar.activation(
                    pt3[:, :, k, :], st[:, :],
                    mybir.ActivationFunctionType.Exp,
                    scale=SCALE,
                )

            def emit_av_step(h, g):
                # g-th global AV k-step for head h (g = 16*c + k)
                if not (0 <= g < 16 * NCOMBO):
                    return
                c, k = divmod(g, 16)
                b, qc = combos[c]
                v4 = vtm[b].rearrange("p (h k c) -> p h k c", h=NH, k=NKT)
                pt3 = pt_tiles[c].rearrange("p (h k q) -> p h k q",
                                            h=NH, k=NKT)
                if k == 0:
                    av_tiles[c][h] = avp.tile([128, 512], f32, tag="av",
                                              name=f"av{c}_{h}")
                av = av_tiles[c][h]
                nc.tensor.matmul(
                    av[0:VSLOT, :], v4[:, h, k, :], pt3[:, h, k, :],
                    start=(k == 0), stop=(k == NKT - 1),
                )
                if k == NKT - 1:
                    # stage this head's O_raw^T + denom row into the
                    # shared [65, 1024] SBUF tile (frees the PSUM bank)
                    if oraw_tiles[c] is None:
                        oraw_tiles[c] = orp.tile([VSLOT, 1024], f32,
                                                 tag="oraw",
                                                 name=f"oraw{c}")
                    orw = oraw_tiles[c]
                    nc.vector.tensor_copy(
                        orw[:, h * 512:(h + 1) * 512], av[0:VSLOT, :])

            def emit_norm_head(c, h):
                # reciprocal of head h's denominator on the DVE —
                # keeps the chain off ScalarE (the kernel bottleneck).
                # The custom-DVE op needs all APs at the same base
                # partition, so first move oraw row 64 to partition 0
                # with a tiny SBUF->SBUF DMA (free engine-wise).
                orw = oraw_tiles[c]
                cs = slice(h * 512, (h + 1) * 512)
                if h == 0:
                    onorm_tiles[c] = onp.tile([128, 512], bf16,
                                              tag="onorm",
                                              name=f"onorm{c}")
                den = rcp.tile([1, 512], f32, tag="den")
                nc.sync.dma_start(out=den[:, :], in_=orw[DH:VSLOT, cs])
                recip = rcp.tile([1, 512], f32, tag="recip")
                rscr = rcp.tile([1, 512], f32, tag="rscr")
                nc.vector.reciprocal_approx_accurate(
                    recip[:, :], den[:, :], rscr[:, :])
                rb = rcp.tile([DH, 512], f32, tag="rb")
                nc.gpsimd.partition_broadcast(rb[:, :], recip[:, :])
                nc.vector.tensor_tensor(
                    onorm_tiles[c][h * DH:(h + 1) * DH, :],
                    orw[0:DH, cs], rb[:, :],
                    op=mybir.AluOpType.mult,
                )

            def emit_outproj_pair(c, s4, reps=1):
                # both jc halves of output-projection row-tile s4 of
                # combo c, staged to one bf16 tile and a single DMA.
                # reps>1 re-runs the matmuls (start=True overwrites;
                # identical result) purely to keep the PE dense in
                # ScalarE-bound combos so HAM never re-throttles.
                b, qc = combos[c]
                q0 = b * S + qc * 512
                onorm = onorm_tiles[c]
                osb = osp.tile([128, 1024], bf16, tag="outsb",
                               name=f"osb{c}_{s4}")
                for jc in range(2):
                    op = opp.tile([128, 512], f32, tag="op",
                                  name=f"op{c}_{s4}_{jc}")
                    for _ in range(reps):
                        nc.tensor.matmul(
                            op[:, :], onorm[:, s4 * 128:(s4 + 1) * 128],
                            wo[:, jc * 512:(jc + 1) * 512],
                            start=True, stop=True,
                        )
                    nc.vector.tensor_copy(
                        osb[:, jc * 512:(jc + 1) * 512], op[:, :])
                r0 = q0 + s4 * 128
                nc.sync.dma_start(
                    out=outp.ap()[r0:r0 + 128, :],
                    in_=osb[:, :],
                )

            # ---- main pipeline ----
            NSTEP = 16 * NCOMBO
            for step in range(NSTEP + LAG + 12):
                # Emission order within a step = PE execution order.
                # Only av-h0 (216ns, always ready) precedes the scores
                # pair: it fills the st-slot wait without overshooting
                # it.  av-h1, outproj and background work follow the
                # scores pair so they can never delay the exp feed —
                # the exp stream is the kernel's bottleneck.
                emit_av_step(0, step - LAG)
                if step < NSTEP:
                    c, k = divmod(step, 16)
                    emit_scores_exp(c, k)
                emit_av_step(1, step - LAG - 1)
                c2, k2 = divmod(step, 16)
                if 1 <= c2 <= NCOMBO and k2 in (5, 6):
                    emit_norm_head(c2 - 1, k2 - 5)
                if 1 <= c2 <= NCOMBO and 10 <= k2 <= 13:
                    emit_outproj_pair(c2 - 1, k2 - 10)
                if step in bg:
                    bg_item(*bg[step])

    nc.compile()
    return nc


def _prep_inputs(x, Wq, bq, Wk, bk, Wv, bv, Wo, bo):
    bf16 = ml_dtypes.bfloat16
    xT = np.ascontiguousarray(
        np.asarray(x, dtype=np.float32).reshape(T, D).T).astype(bf16)
    in_maps = []
    for c in range(NCORES):
        cs = slice(c * 128, (c + 1) * 128)
        in_maps.append({
            "xT": xT,
            "wq": np.ascontiguousarray(Wq[:, cs]).astype(bf16),
            "wk": np.ascontiguousarray(Wk[:, cs]).astype(bf16),
            "wv": np.ascontiguousarray(Wv[:, cs]).astype(bf16),
            "wo": np.ascontiguousarray(Wo[cs, :]).astype(bf16),
            "bq": np.ascontiguousarray(bq[cs]).reshape(128, 1).astype(np.float32),
            "bk": np.ascontiguousarray(bk[cs]).reshape(128, 1).astype(np.float32),
        })
    return in_maps


def kernel(x, Wq, bq, Wk, bk, Wv, bv, Wo, bo, _trace=False, _results=None):
    from concourse.bass_utils import run_bass_kernel_spmd

    x = np.asarray(x); Wq = np.asarray(Wq); Wk = np.asarray(Wk)
    Wv = np.asarray(Wv); Wo = np.asarray(Wo)
    bq = np.asarray(bq); bk = np.asarray(bk); bv = np.asarray(bv)
    bo = np.asarray(bo)

    if "nc" not in _CACHE:
        _CACHE["nc"] = _build_nc()
    nc = _CACHE["nc"]

    in_maps = _prep_inputs(x, Wq, bq, Wk, bk, Wv, bv, Wo, bo)
    res = run_bass_kernel_spmd(
        nc, in_maps, core_ids=list(range(NCORES)), trace=_trace)
    if _results is not None:
        _results.append(res)

    acc = np.zeros((T, D), dtype=np.float32)
    for c in range(NCORES):
        acc += np.asarray(res.results[c]["outp"], dtype=np.float32)
    acc += bv.astype(np.float32) @ Wo.astype(np.float32) + bo.astype(np.float32)
    return acc.reshape(B, S, D)


# revision 30
# speedup vs baseline: 1.1986x; 1.1986x over previous
"""MHA kernel for Trainium2, 8-core tensor-parallel (2 heads per core).

Problem (hardcoded): x [2, 2048, 1024] fp32, Wq/Wk/Wv/Wo [1024, 1024],
bq/bk/bv/bo [1024], H=16 heads, DH=64.  out = MHA(x).

Sharding: heads are split 8 ways (2 heads = 128 proj columns per core).
Each core computes its heads' attention output and a partial output
projection (row-parallel Wo); the host sums the 8 partials and adds the
closed-form bias terms (bv @ Wo + bo).

Structure: the kernel is ScalarE-bound (softmax exp = 16.8M elems/core
at 1 elem/cycle ~= 147us).  Everything else is emitted as a software
pipeline around the exp stream so the PE never idles in lumps (HAM
throttle) and the QKV/V projections ride in the PE's spare cycles
instead of a serial prologue:

  per global k-tile step s (combo c = s//16, k = s%16):
    - background proj work (kt/qt chunks, V token-tiles) per a static
      schedule with just-in-time deadlines
    - scores pair for (c, k): two K=64 matmuls on PE row-groups 0-1 /
      2-3 (run concurrently), exp on ScalarE -> pt (bf16)
    - AV accumulation steps lagged LAG behind scores (h1 one more);
      ones-column in V gives the softmax denominator in PSUM row 64
    - at combo boundaries: oraw copy, one [1,1024] Ln + Exp for both
      heads' reciprocals, gpsimd broadcast, normalize, then the
      output projection spread 2 MMs/step with bf16 staging + DMA out.
"""

import numpy as np
import ml_dtypes

D = 1024
T = 4096          # B*S tokens
S = 2048
B = 2
NH = 2            # heads per core
DH = 64
NCORES = 8
SCALE = 0.125     # 1/sqrt(DH)
NKT = S // 128    # 16 key tiles per batch
NQC = S // 512    # 4 query chunks per batch
NCK = T // 512    # 8 x^T column chunks
VSLOT = DH + 1    # 65: [V columns | ones column]
NCOMBO = B * NQC  # 8
LAG = 5           # AV trails scores by LAG k-tile steps

_CACHE = {}


def _build_nc(reps=1):
    import concourse.bacc as bacc
    import concourse.mybir as mybir
    import concourse.tile as tile
    from concourse.hw_specs import get_activation_tables as _gat

    # Pin Exp and Ln to the one table set that holds both, so the
    # table-load placement pass emits a single ACT_TABLE_LOAD instead of
    # thrashing between exp_and_others and natural_log every combo.
    def _pinned_tables(arch):
        out = {}
        for k, fns in _gat(arch).items():
            if k != "natural_log_exp_and_others":
                fns = {f for f in fns if f.name not in ("Exp", "Ln")}
            out[k] = fns
        return out
    bacc.get_activation_tables = _pinned_tables

    dt = mybir.dt
    f32, bf16 = dt.float32, dt.bfloat16

    nc = bacc.Bacc("TRN2", target_bir_lowering=False, debug=False,
                   num_devices=NCORES)

    xT = nc.dram_tensor("xT", [D, T], bf16, kind="ExternalInput")
    wq_d = nc.dram_tensor("wq", [D, 128], bf16, kind="ExternalInput")
    wk_d = nc.dram_tensor("wk", [D, 128], bf16, kind="ExternalInput")
    wv_d = nc.dram_tensor("wv", [D, 128], bf16, kind="ExternalInput")
    wo_d = nc.dram_tensor("wo", [128, D], bf16, kind="ExternalInput")
    bq_d = nc.dram_tensor("bq", [128, 1], f32, kind="ExternalInput")
    bk_d = nc.dram_tensor("bk", [128, 1], f32, kind="ExternalInput")
    outp = nc.dram_tensor("outp", [T, D], bf16, kind="ExternalOutput")

    with tile.TileContext(nc) as tc:
      for _rep in range(reps):
        with (
            tc.tile_pool(name="persist", bufs=1) as pp,
            tc.tile_pool(name="pt", bufs=2) as ptp,
            tc.tile_pool(name="onorm", bufs=2) as onp,
            tc.tile_pool(name="oraw", bufs=2) as orp,
            tc.tile_pool(name="recip", bufs=2) as rcp,
            tc.tile_pool(name="outsb", bufs=3) as osp,
            tc.tile_pool(name="st_ps", bufs=2, space="PSUM") as stp,
            tc.tile_pool(name="av_ps", bufs=2, space="PSUM") as avp,
            tc.tile_pool(name="op_ps", bufs=2, space="PSUM") as opp,
        ):
            # ---- weights / biases / x^T d-tiles ----
            # DMA issue on the sync engine is ~650ns per dma_start, so
            # order and granularity matter: x batch-0 chunk 0 first (it
            # gates the whole pipeline), then the weights the prologue
            # needs, then coarse transfers for the rest.
            wq = pp.tile([128, D], bf16, tag="wq")
            wk = pp.tile([128, D], bf16, tag="wk")
            wv = pp.tile([128, D], bf16, tag="wv")
            wo = pp.tile([128, D], bf16, tag="wo")
            bq = pp.tile([128, 1], f32, tag="bq")
            bk = pp.tile([128, 1], f32, tag="bk")
            xt_all = pp.tile([128, 8 * T], bf16, tag="xt")
            xt3 = xt_all.rearrange("p (d t) -> p d t", d=8)
            xt = [xt3[:, d, :] for d in range(8)]
            # Per-d-tile DMAs parallelize across HW queues (a single
            # big DMA runs on one queue at ~150GB/s).  The scalar
            # queue carries NO DMA issues: each dma_start costs ~650ns
            # of queue time and would delay the first exp — the exp
            # stream IS the kernel bottleneck.  Weights ride the
            # (otherwise idle) gpsimd software-DGE; all x loads on sync.
            for w_sb, w_dr in ((wk, wk_d), (wq, wq_d), (wv, wv_d)):
                nc.gpsimd.dma_start(
                    out=w_sb.rearrange("p (t c) -> p t c", c=128),
                    in_=w_dr.ap().rearrange("(t p) c -> p t c", p=128),
                )
            nc.gpsimd.dma_start(out=bk[:, :], in_=bk_d.ap()[:, :])
            nc.gpsimd.dma_start(out=bq[:, :], in_=bq_d.ap()[:, :])
            nc.gpsimd.dma_start(out=wo[:, :], in_=wo_d.ap()[:, :])
            for d in range(8):          # b0 chunk 0 (gates prologue)
                nc.sync.dma_start(out=xt3[:, d, 0:512],
                                  in_=xT.ap()[d * 128:(d + 1) * 128, 0:512])
            for d in range(8):          # rest of b0, 3KB lines
                nc.sync.dma_start(
                    out=xt3[:, d, 512:2048],
                    in_=xT.ap()[d * 128:(d + 1) * 128, 512:2048])
            for d in range(8):          # b1, 4KB lines
                nc.sync.dma_start(out=xt3[:, d, 2048:4096],
                                  in_=xT.ap()[d * 128:(d + 1) * 128, 2048:4096])

            qt = pp.tile([128, T], bf16, tag="qt")
            kt = pp.tile([128, T], bf16, tag="kt")
            wq3 = wq.rearrange("p (t c) -> p t c", c=128)
            wk3 = wk.rearrange("p (t c) -> p t c", c=128)
            wv3 = wv.rearrange("p (t c) -> p t c", c=128)

            vtm = []
            for b in range(B):
                v_sb = pp.tile([128, NH * NKT * VSLOT], bf16, tag=f"v{b}")
                v4 = v_sb.rearrange("p (h k c) -> p h k c", h=NH, k=NKT)
                nc.vector.memset(v4[:, :, :, DH:DH + 1], 1.0)
                vtm.append(v_sb)

            # ---- background-work emitters (projections) ----
            _chunk_ps = {}
            _chunk_seq = [0]

            def emit_proj_half(proj_sb, w3, b_sb, nck, half):
                # half a 512-col chunk of Q^T/K^T (4 of 8 d-steps) —
                # finer background quanta so a chunk never overflows
                # one k-tile step's PE slack and stalls the exp stream
                cs = slice(nck * 512, (nck + 1) * 512)
                key = (id(proj_sb), nck)
                if half == 0:
                    _chunk_seq[0] += 1
                    _chunk_ps[key] = opp.tile([128, 512], f32, tag="op",
                                              name=f"pch{_chunk_seq[0]}")
                ps = _chunk_ps[key]
                for d in range(4 * half, 4 * half + 4):
                    nc.tensor.matmul(
                        ps[:, :], w3[:, d, :], xt[d][:, cs],
                        start=(d == 0), stop=(d == 7),
                    )
                if half == 1:
                    nc.vector.tensor_scalar_add(
                        proj_sb[:, cs], ps[:, :], b_sb[:, :])
                    del _chunk_ps[key]

            def emit_proj_chunk(proj_sb, w3, b_sb, nck):
                emit_proj_half(proj_sb, w3, b_sb, nck, 0)
                emit_proj_half(proj_sb, w3, b_sb, nck, 1)

            def emit_v_tile(b, kti):
                # one token-major V tile [128 tok, 2x64] for batch b
                v4 = vtm[b].rearrange("p (h k c) -> p h k c", h=NH, k=NKT)
                tok0 = b * S + kti * 128
                ps = opp.tile([128, 128], f32, tag="op")
                for d in range(8):
                    nc.tensor.matmul(
                        ps[:, :], xt[d][:, tok0:tok0 + 128], wv3[:, d, :],
                        start=(d == 0), stop=(d == 7),
                    )
                nc.vector.tensor_copy(
                    v4[:, :, kti, 0:DH],
                    ps.rearrange("p (h c) -> p h c", h=NH)[:, :, :],
                )

            def bg_item(kind, a, bb):
                if kind == "kt":
                    emit_proj_chunk(kt, wk3, bk, a)
                elif kind == "qt":
                    emit_proj_chunk(qt, wq3, bq, a)
                elif kind == "kth":
                    emit_proj_half(kt, wk3, bk, a, bb)
                elif kind == "qth":
                    emit_proj_half(qt, wq3, bq, a, bb)
                else:
                    emit_v_tile(bb, a)

            # static schedule: step -> background item.  Deadlines:
            # scores(c,k) needs kt chunk (4b + k//4) and qt chunk
            # (4b + qc) by step 16c+k; av(c,k) at step 16c+k+LAG needs
            # v(b,k).  At most ONE item per step and none at k<3
            # (except combo 0) so the in-order PE never delays a
            # scores pair by more than the pipeline slack -> the exp
            # stream on ScalarE (the bottleneck) never starves.
            bg = {}
            def at(step, kind, a, bb=0):
                assert step not in bg, step
                bg[step] = (kind, a, bb)
            # Slots avoid outproj steps (k 10-13) and combo boundaries
            # (k 0-2) so a background chunk never delays a scores pair
            # past the exp stream's ~0.8us of ring slack.  Exceptions
            # are small v-tiles whose AV deadlines force early steps.
            at(1, "kt", 1); at(4, "kt", 2); at(7, "kt", 3)
            at(14, "qt", 1)                      # b0 qc1, due 16
            vi = 0
            for s in list(range(16)) + [16, 17, 18, 19]:
                if s not in bg and vi < NKT:
                    at(s, "v", vi, 0); vi += 1    # v(b0,k) due k+LAG
            at(20, "qth", 2, 0); at(21, "qth", 2, 1)   # b0 qc2, due 32
            at(35, "kth", 4, 0); at(36, "kth", 4, 1)   # kt b1 j0, due 64
            at(37, "kth", 5, 0); at(38, "kth", 5, 1)   # j1, due 68
            at(39, "qth", 3, 0); at(40, "qth", 3, 1)   # b0 qc3, due 48
            at(41, "v", 0, 1); at(46, "v", 1, 1); at(47, "v", 2, 1)
            at(51, "kth", 6, 0); at(52, "kth", 6, 1)   # j2, due 72
            at(53, "kth", 7, 0); at(54, "kth", 7, 1)   # j3, due 76
            at(55, "qth", 4, 0); at(56, "qth", 4, 1)   # b1 qc0, due 64
            at(57, "v", 3, 1); at(62, "v", 4, 1); at(63, "v", 5, 1)
            at(67, "qth", 5, 0); at(68, "qth", 5, 1)   # b1 qc1, due 80
            for i, s in enumerate((69, 70, 71, 72, 73, 78, 79)):
                at(s, "v", 6 + i, 1)             # v b1 6-12, due 75+
            at(81, "v", 13, 1); at(82, "v", 14, 1)     # due 82/83
            at(83, "v", 15, 1)                   # due 84
            at(84, "qth", 6, 0); at(85, "qth", 6, 1)   # due 96
            at(99, "qth", 7, 0); at(100, "qth", 7, 1)  # due 112
            # prologue: minimum to start combo 0
            emit_proj_chunk(kt, wk3, bk, 0)
            emit_proj_chunk(qt, wq3, bq, 0)

            # ---- attention pipeline state ----
            combos = [(b, qc) for b in range(B) for qc in range(NQC)]
            pt_tiles = [None] * NCOMBO          # [128, NH*NKT*512] bf16
            av_tiles = [[None, None] for _ in range(NCOMBO)]
            oraw_tiles = [None] * NCOMBO        # [VSLOT, 1024] f32 SBUF
            onorm_tiles = [None] * NCOMBO

            def emit_scores_exp(c, k):
                b, qc = combos[c]
                q0 = b * S + qc * 512
                k0 = b * S + k * 128
                if k == 0:
                    pt_tiles[c] = ptp.tile([128, NH * NKT * 512], bf16,
                                           tag="pt", name=f"pt{c}")
                pt3 = pt_tiles[c].rearrange("p (h k q) -> p h k q",
                                            h=NH, k=NKT)
                st = stp.tile([128, 1024], f32, tag="st")
                for h in range(NH):
                    hp = h * DH
                    nc.tensor.matmul(
                        st[:, h * 512:(h + 1) * 512],
                        kt[hp:hp + DH, k0:k0 + 128],
                        qt[hp:hp + DH, q0:q0 + 512],
                        start=True, stop=True,
                    )
                nc.scalar.activation(
                    pt3[:, :, k, :], st[:, :],
                    mybir.ActivationFunctionType.Exp,
                    scale=SCALE,
                )

            def emit_av_step(h, g):
                # g-th global AV k-step for head h (g = 16*c + k)
                if not (0 <= g < 16 * NCOMBO):
                    return
                c, k = divmod(g, 16)
                b, qc = combos[c]
                v4 = vtm[b].rearrange("p (h k c) -> p h k c", h=NH, k=NKT)
                pt3 = pt_tiles[c].rearrange("p (h k q) -> p h k q",
                                            h=NH, k=NKT)
                if k == 0:
                    av_tiles[c][h] = avp.tile([128, 512], f32, tag="av",
                                              name=f"av{c}_{h}")
                av = av_tiles[c][h]
                nc.tensor.matmul(
                    av[0:VSLOT, :], v4[:, h, k, :], pt3[:, h, k, :],
                    start=(k == 0), stop=(k == NKT - 1),
                )
                if k == NKT - 1:
                    # stage this head's O_raw^T + denom row into the
                    # shared [65, 1024] SBUF tile (frees the PSUM bank)
                    if oraw_tiles[c] is None:
                        oraw_tiles[c] = orp.tile([VSLOT, 1024], f32,
                                                 tag="oraw",
                                                 name=f"oraw{c}")
                    orw = oraw_tiles[c]
                    nc.vector.tensor_copy(
                        orw[:, h * 512:(h + 1) * 512], av[0:VSLOT, :])

            def emit_norm_head(c, h):
                # reciprocal of head h's denominator on the DVE —
                # keeps the chain off ScalarE (the kernel bottleneck).
                # The custom-DVE op needs all APs at the same base
                # partition, so first move oraw row 64 to partition 0
                # with a tiny SBUF->SBUF DMA (free engine-wise).
                orw = oraw_tiles[c]
                cs = slice(h * 512, (h + 1) * 512)
                if h == 0:
                    onorm_tiles[c] = onp.tile([128, 512], bf16,
                                              tag="onorm",
                                              name=f"onorm{c}")
                den = rcp.tile([1, 512], f32, tag="den")
                nc.sync.dma_start(out=den[:, :], in_=orw[DH:VSLOT, cs])
                recip = rcp.tile([1, 512], f32, tag="recip")
                rscr = rcp.tile([1, 512], f32, tag="rscr")
                nc.vector.reciprocal_approx_accurate(
                    recip[:, :], den[:, :], rscr[:, :])
                rb = rcp.tile([DH, 512], f32, tag="rb")
                nc.gpsimd.partition_broadcast(rb[:, :], recip[:, :])
                nc.vector.tensor_tensor(
                    onorm_tiles[c][h * DH:(h + 1) * DH, :],
                    orw[0:DH, cs], rb[:, :],
                    op=mybir.AluOpType.mult,
                )

            def emit_outproj_pair(c, s4, reps=1):
                # both jc halves of output-projection row-tile s4 of
                # combo c, staged to one bf16 tile and a single DMA.
                # reps>1 re-runs the matmuls (start=True overwrites;
                # identical result) purely to keep the PE dense in
                # ScalarE-bound combos so HAM never re-throttles.
                b, qc = combos[c]
                q0 = b * S + qc * 512
                onorm = onorm_tiles[c]
                osb = osp.tile([128, 1024], bf16, tag="outsb",
                               name=f"osb{c}_{s4}")
                for jc in range(2):
                    op = opp.tile([128, 512], f32, tag="op",
                                  name=f"op{c}_{s4}_{jc}")
                    for _ in range(reps):
                        nc.tensor.matmul(
                            op[:, :], onorm[:, s4 * 128:(s4 + 1) * 128],
                            wo[:, jc * 512:(jc + 1) * 512],
                            start=True, stop=True,
                        )
                    nc.vector.tensor_copy(
                        osb[:, jc * 512:(jc + 1) * 512], op[:, :])
                r0 = q0 + s4 * 128
                nc.sync.dma_start(
                    out=outp.ap()[r0:r0 + 128, :],
                    in_=osb[:, :],
                )

            # ---- main pipeline ----
            NSTEP = 16 * NCOMBO
            for step in range(NSTEP + LAG + 12):
                if step in bg:
                    bg_item(*bg[step])
                if step < NSTEP:
                    c, k = divmod(step, 16)
                    emit_scores_exp(c, k)
                emit_av_step(0, step - LAG)
                emit_av_step(1, step - LAG - 1)
                c2, k2 = divmod(step, 16)
                if 1 <= c2 <= NCOMBO and k2 in (5, 6):
                    emit_norm_head(c2 - 1, k2 - 5)
                if 1 <= c2 <= NCOMBO and 10 <= k2 <= 13:
                    emit_outproj_pair(c2 - 1, k2 - 10)

    nc.compile()
    return nc


def _prep_inputs(x, Wq, bq, Wk, bk, Wv, bv, Wo, bo):
    bf16 = ml_dtypes.bfloat16
    xT = np.ascontiguousarray(
        np.asarray(x, dtype=np.float32).reshape(T, D).T).astype(bf16)
    in_maps = []
    for c in range(NCORES):
        cs = slice(c * 128, (c + 1) * 128)
        in_maps.append({
            "xT": xT,
            "wq": np.ascontiguousarray(Wq[:, cs]).astype(bf16),
            "wk": np.ascontiguousarray(Wk[:, cs]).astype(bf16),
            "wv": np.ascontiguousarray(Wv[:, cs]).astype(bf16),
            "wo": np.ascontiguousarray(Wo[cs, :]).astype(bf16),
            "bq": np.ascontiguousarray(bq[cs]).reshape(128, 1).astype(np.float32),
            "bk": np.ascontiguousarray(bk[cs]).reshape(128, 1).astype(np.float32),
        })
    return in_maps


def kernel(x, Wq, bq, Wk, bk, Wv, bv, Wo, bo, _trace=False, _results=None):
    from concourse.bass_utils import run_bass_kernel_spmd

    x = np.asarray(x); Wq = np.asarray(Wq); Wk = np.asarray(Wk)
    Wv = np.asarray(Wv); Wo = np.asarray(Wo)
    bq = np.asarray(bq); bk = np.asarray(bk); bv = np.asarray(bv)
    bo = np.asarray(bo)

    if "nc" not in _CACHE:
        _CACHE["nc"] = _build_nc()
    nc = _CACHE["nc"]

    in_maps = _prep_inputs(x, Wq, bq, Wk, bk, Wv, bv, Wo, bo)
    res = run_bass_kernel_spmd(
        nc, in_maps, core_ids=list(range(NCORES)), trace=_trace)
    if _results is not None:
        _results.append(res)

    acc = np.zeros((T, D), dtype=np.float32)
    for c in range(NCORES):
        acc += np.asarray(res.results[c]["outp"], dtype=np.float32)
    acc += bv.astype(np.float32) @ Wo.astype(np.float32) + bo.astype(np.float32)
    return acc.reshape(B, S, D)
